# revision 20
# baseline (speedup 1.0000x reference)
"""MK-MMD loss kernel for Trainium2 (8 NeuronCores, data-parallel).

Reference computation:
    pairs (x,x',y,y') from consecutive rows of Xs/Xt
    D1=|x-x'|^2, D2=|y-y'|^2, D3=|x-y'|^2, D4=|x'-y|^2
    h_u[g,p] = exp(-D1*s_g) + exp(-D2*s_g) - exp(-D3*s_g) - exp(-D4*s_g)
    out = betas^T @ mean_p(h_u)          -> shape (1,)

Strategy (memory-bound: 512MB of X data streamed once):
  - Shard samples across 8 cores (16384 rows of each of Xs/Xt per core).
  - A custom DVE ucode op SQDIFF_REDUCE computes out=(a-b)^2 with a fused
    free-dim sum (accum_out), so each of the four pair distances is ONE
    VectorE instruction per 128 pairs. No norms/dots/fixups; ScalarE only
    runs the 29 exp ops of the gamma stage at the end.
  - Per-pair D values land in a [128, 256] column buffer; the tail stage
    computes exp(-s_g * D) (ACT, one op per gamma) and signed sums
    (VectorE scalar_tensor_tensor subtract + accum), emitting [128, 29]
    per-partition partials per core.
  - Host: sum partials over partitions/cores, divide by pair count, dot
    betas.
"""

import numpy as np

import concourse.bacc as bacc
import concourse.tile as tile
from concourse import dve_ops, mybir
from concourse.bass_utils import run_bass_kernel_spmd
from concourse.dve_spec import (
    AluOp as DALU,
    Spec,
    Src0,
    Src1,
    _has_src1,
    lower,
    scan as dve_scan,
    sq,
)
from concourse.dve_uop import DveOpSpec

N_SAMPLES = 131072
N_FEAT = 512
N_KERNELS = 29
N_CORES = 8

ROWS_PER_CORE = N_SAMPLES // N_CORES        # 16384 samples per core
PAIRS_PER_CORE = ROWS_PER_CORE // 2         # 8192
# View each core's [16384, 512] shard as [4096, 2048]: one row = 2 pairs,
# each pair = 1024 contiguous floats. A [128, 2048] tile = 256 pairs,
# fully contiguous 1MB DMA.
PAIRS_PER_ROW = 8                           # DRAM view row = 8 pairs
ROW_W = 1024 * PAIRS_PER_ROW                # 4096 floats per view row
QROWS = ROWS_PER_CORE // (2 * PAIRS_PER_ROW)  # 2048
N_ITERS = QROWS // 128                      # 16 tiles per core
N_QUARTERS = 4                              # gamma stage chunks
TILES_PER_QUARTER = N_ITERS // N_QUARTERS   # 4
SUBPAIRS_PER_TILE = PAIRS_PER_ROW           # 4 subpair columns per tile
QCOLS = 4 * SUBPAIRS_PER_TILE * TILES_PER_QUARTER  # 64 D-columns per quarter

F32 = mybir.dt.float32
ALU = mybir.AluOpType
ACTF = mybir.ActivationFunctionType


def _make_sqdiff_scan_op():
    """Register the SQDIFF_SCAN custom DVE op: out = running prefix sum of
    (in0-in1)^2 along the free dim. Segment sums are recovered by sampling
    and differencing prefix columns. Idempotent."""
    name = "SQDIFF_SCAN"
    for op in dve_ops.OPS:
        if op.name == name:
            return op

    def _ref(in0, in1, c0, c1, c2):
        b = ((in0.astype(np.float32) - in1) ** 2).astype(np.float32)
        return np.cumsum(b.reshape(b.shape[0], -1), axis=-1, dtype=np.float32)

    spec = Spec(body=dve_scan(DALU.ADD, sq(Src0 - Src1)), reference=_ref)
    opcode = max(dve_ops._SUB_OPCODE_FOR_NAME.values()) + 1
    assert opcode < 0x20, "custom DVE opcode rows exhausted"
    dve_ops._SUB_OPCODE_FOR_NAME[name] = opcode
    shas = {
        ver: DveOpSpec(
            name=name,
            opcode=opcode,
            uops=lower(spec, ver=ver),
            rd1_en=_has_src1(spec),
        ).sha(ver)
        for ver in ("v3", "v4")
    }
    op = dve_ops.DveOp(name, spec, subdim=False, uops_sha=shas)
    dve_ops.OPS.append(op)
    dve_ops.CUSTOM_DVE_SPECS[op.name] = op.spec
    return op


SQDIFF_SCAN = _make_sqdiff_scan_op()


def _gamma_scales() -> np.ndarray:
    gammas = np.power(
        np.float32(2.0), np.arange(-3.5, 3.75, 0.25, dtype=np.float32)
    ).astype(np.float32)
    return (1.0 / (2.0 * gammas * gammas)).astype(np.float32)


def _build():
    nc = bacc.Bacc("TRN2", target_bir_lowering=False, debug=False)

    xs_d = nc.dram_tensor("Xs", [QROWS, ROW_W], F32, kind="ExternalInput").ap()
    xt_d = nc.dram_tensor("Xt", [QROWS, ROW_W], F32, kind="ExternalInput").ap()
    # One row of per-(gamma, D-column) exp sums (already reduced over the 128
    # pair-partitions by TensorE) per quarter of the main loop.
    r_d = nc.dram_tensor(
        "R", [N_QUARTERS, N_KERNELS * QCOLS], F32, kind="ExternalOutput"
    ).ap()

    inv2g2 = _gamma_scales()

    with tile.TileContext(nc) as tc:
        with (
            tc.tile_pool(name="xin", bufs=2) as xin_pool,
            tc.tile_pool(name="acc", bufs=1) as acc_pool,
            tc.tile_pool(name="sdump", bufs=2) as sdump_pool,
            tc.tile_pool(name="ebuf", bufs=2) as ebuf_pool,
            tc.tile_pool(name="psum", bufs=2, space="PSUM") as psum_pool,
        ):
            # Per-quarter D buffers (separate tiles so the gamma stage for
            # quarter q only depends on its own main-loop tiles).
            # Column layout (interleaved): col = c_local*4 + d, where d is the
            # distance index (D1,D2 positive / D3,D4 negative).
            dqs = [
                acc_pool.tile([128, QCOLS], F32, name=f"dq{q}")
                for q in range(N_QUARTERS)
            ]
            ones_v = acc_pool.tile([128, 1], F32)
            nc.gpsimd.memset(ones_v[:], 1.0)

            def gamma_quarter(q):
                dq = dqs[q]
                ebuf = ebuf_pool.tile(
                    [128, N_KERNELS * QCOLS], F32, tag="ebuf", name=f"ebuf{q}"
                )
                for g in range(N_KERNELS):
                    nc.scalar.activation(
                        ebuf[:, g * QCOLS : (g + 1) * QCOLS],
                        dq[:],
                        ACTF.Exp,
                        scale=float(-inv2g2[g]),
                    )
                psum = psum_pool.tile(
                    [1, N_KERNELS * QCOLS], F32, tag="psum", name=f"psum{q}"
                )
                ncols = N_KERNELS * QCOLS
                for j in range(0, ncols, 512):
                    e = min(j + 512, ncols)
                    nc.tensor.matmul(
                        psum[:, j:e], ones_v[:], ebuf[:, j:e], start=True, stop=True
                    )
                rrow = ebuf_pool.tile(
                    [1, N_KERNELS * QCOLS], F32, tag="rrow", name=f"rrow{q}"
                )
                nc.scalar.copy(rrow[:], psum[:])
                # Issue from gpsimd (separate DMA queue) so this result store
                # can't head-of-line-block the sync-queue input stream.
                nc.gpsimd.dma_start(out=r_d[q : q + 1, :], in_=rrow[:])

            U = SUBPAIRS_PER_TILE
            for t in range(N_ITERS):
                q, tl = divmod(t, TILES_PER_QUARTER)
                xs = xin_pool.tile([128, ROW_W], F32, tag="xs")
                nc.default_dma_engine.dma_start(
                    out=xs[:], in_=xs_d[t * 128 : (t + 1) * 128, :]
                )
                xt = xin_pool.tile([128, ROW_W], F32, tag="xt")
                nc.default_dma_engine.dma_start(
                    out=xt[:], in_=xt_d[t * 128 : (t + 1) * 128, :]
                )
                # Strided 4-block views: the even/odd sample halves of each of
                # the U=4 subpairs in this tile. [128, U, 512]
                xs_b = xs.rearrange("p (u h) -> p u h", h=1024)
                xt_b = xt.rearrange("p (u h) -> p u h", h=1024)
                a = xs_b[:, :, 0:512]
                b = xs_b[:, :, 512:1024]
                yc = xt_b[:, :, 0:512]
                yd = xt_b[:, :, 512:1024]
                dq = dqs[q]
                dq_v = dq.rearrange("p (cc four) -> p cc four", four=4)
                for d, (i0, i1) in enumerate(
                    ((a, b), (yc, yd), (a, yd), (b, yc))
                ):
                    # One scan per distance: prefix sums of (i0-i1)^2 over the
                    # concatenated U segments of 512.
                    sdump = sdump_pool.tile(
                        [128, U * 512], F32, tag="sdump", name=f"sd{t}_{d}"
                    )
                    nc.vector._custom_dve(SQDIFF_SCAN, out=sdump[:], in0=i0, in1=i1)
                    # Segment sums: S[511] and S[512(u+1)-1]-S[512u-1].
                    samples = sdump.rearrange("p (u h) -> p u h", h=512)[
                        :, :, 511:512
                    ]  # [128, U, 1]
                    c0 = U * tl
                    nc.vector.tensor_copy(
                        dq_v[:, c0 : c0 + 1, d : d + 1], samples[:, 0:1, :]
                    )
                    nc.vector.tensor_sub(
                        dq_v[:, c0 + 1 : c0 + U, d : d + 1],
                        samples[:, 1:U, :],
                        samples[:, 0 : U - 1, :],
                    )
                if tl == TILES_PER_QUARTER - 1:
                    gamma_quarter(q)

    nc.compile()
    return nc


_NC_CACHE = None


def _get_nc():
    global _NC_CACHE
    if _NC_CACHE is None:
        _NC_CACHE = _build()
    return _NC_CACHE


def _make_in_maps(Xs: np.ndarray, Xt: np.ndarray):
    in_maps = []
    for i in range(N_CORES):
        sl = slice(i * ROWS_PER_CORE, (i + 1) * ROWS_PER_CORE)
        in_maps.append(
            {
                "Xs": np.ascontiguousarray(Xs[sl]).reshape(QROWS, ROW_W),
                "Xt": np.ascontiguousarray(Xt[sl]).reshape(QROWS, ROW_W),
            }
        )
    return in_maps


_SIGNS = np.array([1.0, 1.0, -1.0, -1.0])  # D1, D2 positive; D3, D4 negative


def _finish(results, betas: np.ndarray) -> np.ndarray:
    tot = np.zeros(N_KERNELS, dtype=np.float64)
    for r in results:
        # [quarters, kernels, c_local, d] exp sums (pre-reduced over pairs)
        e = r["R"].astype(np.float64).reshape(
            N_QUARTERS, N_KERNELS, QCOLS // 4, 4
        )
        tot += (e * _SIGNS).sum(axis=(0, 2, 3))
    hat_d = tot / (N_SAMPLES // 2)
    out = betas.astype(np.float64).reshape(N_KERNELS) @ hat_d
    return np.array([out], dtype=np.float32)


def run(Xs, Xt, betas, **spmd_kwargs):
    """Run the device kernel; returns (output, BassKernelResults)."""
    nc = _get_nc()
    in_maps = _make_in_maps(np.asarray(Xs), np.asarray(Xt))
    res = run_bass_kernel_spmd(nc, in_maps, list(range(N_CORES)), **spmd_kwargs)
    return _finish(res.results, np.asarray(betas)), res


def kernel(Xs, Xt, betas):
    out, _ = run(Xs, Xt, betas)
    return out
